# revision 28
# baseline (speedup 1.0000x reference)
"""Bayesian uncertainty distance kernel for TRN2 (8 NeuronCores, SPMD).

Math (per reference):
    W_s  = weight_mu + eps_w[s] * softplus(weight_rho)          [S,D,D]
    b_s  = bias_mu   + eps_b[s] * softplus(bias_rho)            [S,D]
    qt_s = query @ W_s + b_s                                    [S,Q,D]
    d2_s = ||qt_s||^2 - 2 qt_s.proto^T + ||proto||^2            [S,Q,P]
    mean = mean_s sqrt(d2_s);  std = std_s(sqrt(d2_s), ddof=1)

Sharding: data-parallel over Q (8192 -> 8 x 1024). Everything else replicated.

v3 design (per core, Q=1024, P=2048, D=256, S=10):
  - samples are DEFINED as x8_s := fp8e4(-2*(query@W_s + b_s)); all moments
    are derived consistently from these fp8 values so rounding cancels in
    the variance to first order.  prototypes y8 := fp8e4(proto); the pn
    norm and the fp16 copy yT16 are derived FROM y8 for exact consistency.
  - phase 1 per s: fp16 qt matmuls -> DVE tensor_scalar writes x8 directly
    (psum*-2 + -2b, fp8 out); x16u = up(x8) exact; x2 = x16u^2 (DVE fp16);
    qn columns via width-1 PE matmuls (lhsT=x2 128-col slices, rhs=ones col)
    accumulated into one [128, 80] psum tile; xsum psum += eye8 @ x8_s.
  - phase 2 per (qtile, phalf): per s a [128,1024] psum tile gets
    pn via a DoubleRow fp8 seed (ones8x [1,2,128] x pn8x [1,2,P], value
    2*fp8(pn/2)) + the cross term via DoubleRow fp8 matmuls (K=256 in one
    instr, 0.5 cyc/col); dist_s = ACT Sqrt(psum + qn_s column bias) ->
    fp32 SBUF; the s-sum runs on the PE as identity-matmul accumulation
    with float32r (1 cyc/col, fp22 truncation compensated by COMP scale)
    instead of DVE fp32 adds (1x-mode, was the 165us DVE bottleneck).
  - variance via sum-of-d2: ss psum = ones x 10pn16 + xsum16 @ yT16 (fp16);
    u = ss - macc^2*COMP^2/10 (DVE stt); std = Sqrt(u/(S-1) + qnsum/(S-1))
    with the qnsum column as the ACT bias; mean = macc*COMP/10 (DVE ts).

The host does only O(S*D^2) prep in numpy (softplus, W_s, transposes, pn).
"""

import os
import numpy as np
import ml_dtypes

import concourse.bass as bass
import concourse.mybir as mybir
import concourse.tile as tile
from concourse import bacc, bass_utils

AF = mybir.ActivationFunctionType
ALU = mybir.AluOpType
DR = mybir.MatmulPerfMode.DoubleRow

F32 = mybir.dt.float32
F32R = mybir.dt.float32r
F16 = mybir.dt.float16
F8 = mybir.dt.float8e4
NP_F8 = ml_dtypes.float8_e4m3  # TRN float8e4: max normal +-240, has inf

NCORES = 8
D = 256
Q_FULL = 8192
P = 2048
S = 10
QLOC = Q_FULL // NCORES  # 1024
ET = D // 128  # 2 e-tiles
DT = D // 128  # 2 d-tiles
QT = QLOC // 128  # 8 q-tiles per core
PH = 2048  # phase-2 psum tile width (4 banks)
NPH = P // PH  # 1

# fp22 truncation compensation: the PE reads float32r by truncating the
# mantissa to 13 bits, losing an average of 2^-14 relative on the positive
# dist values; COMP re-centres macc (validated against the reference).
COMP = 1.0

_CACHE = {}
LAST_RESULTS = None


def _build_bass():
    nc = bacc.Bacc(
        "TRN2",
        target_bir_lowering=False,
        debug=False,
        num_devices=NCORES,
    )
    ins = {}
    ins["qT16"] = nc.dram_tensor("qT16", [128, DT * QLOC], F16, kind="ExternalInput").ap()
    ins["W16"] = nc.dram_tensor("W16", [S, 128, DT * 256], F16, kind="ExternalInput").ap()
    ins["b2T"] = nc.dram_tensor("b2T", [128, ET * S], F32, kind="ExternalInput").ap()
    ins["y8"] = nc.dram_tensor("y8", [128, ET, P], F8, kind="ExternalInput").ap()
    ins["yT16"] = nc.dram_tensor("yT16", [128, ET, P], F16, kind="ExternalInput").ap()
    ins["pn16q"] = nc.dram_tensor("pn16q", [1, P], F16, kind="ExternalInput").ap()
    ins["pn10_16"] = nc.dram_tensor("pn10_16", [1, P], F16, kind="ExternalInput").ap()
    ins["onesr16"] = nc.dram_tensor("onesr16", [1, 128], F16, kind="ExternalInput").ap()
    ins["o16c"] = nc.dram_tensor("o16c", [128, 1], F16, kind="ExternalInput").ap()
    ins["eye8"] = nc.dram_tensor("eye8", [128, 128], F8, kind="ExternalInput").ap()
    mean_o = nc.dram_tensor("mean_o", [QLOC, P], F32, kind="ExternalOutput").ap()
    std_o = nc.dram_tensor("std_o", [QLOC, P], F32, kind="ExternalOutput").ap()

    with tile.TileContext(nc) as tc:
        _kernel_body(tc, ins, mean_o, std_o)
    nc.compile()
    return nc


def _kernel_body(tc, ins, mean_o, std_o):
    nc = tc.nc
    from contextlib import ExitStack

    ctx = ExitStack()
    with ctx:
        cpool = ctx.enter_context(tc.tile_pool(name="consts", bufs=1))
        wpool = ctx.enter_context(tc.tile_pool(name="wpool", bufs=2))
        x8pool = ctx.enter_context(tc.tile_pool(name="x8pool", bufs=S))
        xlopool = ctx.enter_context(tc.tile_pool(name="xlopool", bufs=S))
        x16pool = ctx.enter_context(tc.tile_pool(name="x16p", bufs=2))
        x16upool = ctx.enter_context(tc.tile_pool(name="x16u", bufs=2))
        x2pool = ctx.enter_context(tc.tile_pool(name="x2pool", bufs=2))
        xsumpool = ctx.enter_context(tc.tile_pool(name="xsumpool", bufs=1))
        qnpool = ctx.enter_context(tc.tile_pool(name="qnpool", bufs=1))
        distpool = ctx.enter_context(tc.tile_pool(name="distpool", bufs=2))
        finpool = ctx.enter_context(tc.tile_pool(name="finpool", bufs=2))
        outpool = ctx.enter_context(tc.tile_pool(name="outpool", bufs=3))

        # ---- constants into SBUF ----
        qT_t = cpool.tile([128, DT * QLOC], F16)
        nc.sync.dma_start(qT_t[:], ins["qT16"])
        b2_t = cpool.tile([128, ET * S], F32)
        nc.sync.dma_start(b2_t[:], ins["b2T"])
        y8_t = cpool.tile([128, ET, P], F8)
        nc.sync.dma_start(y8_t[:], ins["y8"])
        yT16_t = cpool.tile([128, ET, P], F16)
        nc.sync.dma_start(yT16_t[:], ins["yT16"])
        pn16q_t = cpool.tile([1, P], F16)
        nc.sync.dma_start(pn16q_t[:], ins["pn16q"])
        pn10_t = cpool.tile([1, P], F16)
        nc.sync.dma_start(pn10_t[:], ins["pn10_16"])
        onesr16_t = cpool.tile([1, 128], F16)
        nc.sync.dma_start(onesr16_t[:], ins["onesr16"])
        o16c_t = cpool.tile([128, 1], F16)
        nc.sync.dma_start(o16c_t[:], ins["o16c"])
        eye8_t = cpool.tile([128, 128], F8)
        nc.sync.dma_start(eye8_t[:], ins["eye8"])

        xsum16_t = xsumpool.tile([128, ET, QLOC], F16)
        # qn columns: [128, QT, S] fp32; [128,1] slices feed the ACT Sqrt bias
        qncol_t = qnpool.tile([128, QT, S], F32)
        qn9r_t = qnpool.tile([128, QT], F32)
        qn9_t = qnpool.tile([128, QT], F32)  # qnsum/(S-1) bias columns for std

        x_tiles = []
        # ---------- phase 1: per-sample fp8 transformed queries ----------
        with tc.tile_pool(name="pp1", bufs=2, space="PSUM") as pp1, \
             tc.tile_pool(name="ppqn", bufs=1, space="PSUM") as ppqn, \
             tc.tile_pool(name="ppxs", bufs=1, space="PSUM") as ppxs:
            qncolp = ppqn.tile([128, QT * S], F32)
            xsump = ppxs.tile([128, ET * QLOC], F32)
            for s in range(S):
                w_t = wpool.tile([128, DT * 256], F16, tag="w")
                nc.sync.dma_start(w_t[:], ins["W16"][s])
                x16_s = x16pool.tile([128, ET, QLOC], F16, tag="x16", name=f"x16_{s}")
                xhi_s = x8pool.tile([128, ET, QLOC], F8, tag="x", name=f"xh{s}")
                xlo_s = xlopool.tile([128, ET, QLOC], F8, tag="xl", name=f"xl{s}")
                x_tiles.append((xhi_s, xlo_s))
                for et in range(ET):
                    for qc in range(2):
                        qp = pp1.tile([128, 512], F32, tag="ps", name=f"qp{s}_{et}_{qc}")
                        for dt_ in range(DT):
                            nc.tensor.matmul(
                                qp[:],
                                lhsT=w_t[:, dt_ * 256 + et * 128 : dt_ * 256 + et * 128 + 128],
                                rhs=qT_t[:, dt_ * QLOC + qc * 512 : dt_ * QLOC + qc * 512 + 512],
                                start=(dt_ == 0),
                                stop=(dt_ == DT - 1),
                            )
                        # x16 = fp16(-2*qt - 2*b) from psum on DVE
                        nc.vector.tensor_scalar(
                            x16_s[:, et, qc * 512 : qc * 512 + 512],
                            qp[:],
                            -2.0,
                            b2_t[:, et * S + s : et * S + s + 1],
                            ALU.mult,
                            ALU.add,
                        )
                # two-term fp8 split: x ~= xhi + xlo with ~2^-9 relative
                # residual, so the DoubleRow cross loses almost no precision
                nc.vector.tensor_copy(xhi_s[:], x16_s[:])
                x16u = x16upool.tile([128, ET, QLOC], F16, tag="xu", name=f"xu{s}")
                nc.vector.tensor_copy(x16u[:], xhi_s[:])
                nc.vector.tensor_tensor(xlo_s[:], x16_s[:], x16u[:], ALU.subtract)
                x2_s = x2pool.tile([128, ET, QLOC], F16, tag="x2", name=f"x2_{s}")
                nc.vector.tensor_tensor(x2_s[:], x16_s[:], x16_s[:], ALU.mult)
                # qn columns: width-1 matmuls, one column per (qtile, s)
                for qt8 in range(QT):
                    for et in range(ET):
                        nc.tensor.matmul(
                            qncolp[:, qt8 * S + s : qt8 * S + s + 1],
                            lhsT=x2_s[:, et, qt8 * 128 : qt8 * 128 + 128],
                            rhs=o16c_t[:],
                            start=(et == 0),
                            stop=(et == ET - 1),
                            skip_group_check=True,
                        )
                # xsum += xhi_s + xlo_s (exact: eye8 matmuls, psum fp32)
                for et in range(ET):
                    for qc in range(2):
                        for half in (xhi_s, xlo_s):
                            nc.tensor.matmul(
                                xsump[:, et * QLOC + qc * 512 : et * QLOC + qc * 512 + 512],
                                lhsT=eye8_t[:],
                                rhs=half[:, et, qc * 512 : qc * 512 + 512],
                                start=(s == 0 and half is xhi_s),
                                stop=(s == S - 1 and half is xlo_s),
                                skip_group_check=True,
                            )
            # qn = 0.25 * sum x^2   (x = -2(qt+b))
            nc.vector.tensor_scalar_mul(
                qncol_t[:].rearrange("p a b -> p (a b)"), qncolp[:], 0.25
            )
            # qnsum/(S-1) columns for the std bias (qncol already has the 0.25)
            nc.vector.tensor_reduce(
                qn9r_t[:], qncol_t[:], axis=mybir.AxisListType.X, op=ALU.add
            )
            nc.vector.tensor_scalar_mul(qn9_t[:], qn9r_t[:], 1.0 / (S - 1))
            nc.vector.tensor_copy(
                xsum16_t[:].rearrange("p a b -> p (a b)"), xsump[:]
            )

        # ---------- phase 2: distances, moments, outputs ----------
        with tc.tile_pool(name="ppC", bufs=2, space="PSUM") as ppC, \
             tc.tile_pool(name="maccpool", bufs=2) as maccpool:
            for qt8 in range(QT):
                for ph in range(NPH):
                    macc_t = maccpool.tile([128, PH], F32, tag="macc", name=f"m{qt8}_{ph}")
                    for s in range(S):
                        cp = ppC.tile([128, PH], F32, tag="ps", name=f"c{qt8}_{ph}_{s}")
                        for c in range(PH // 512):
                            o = ph * PH + c * 512
                            # pn seed: rank-1 ones x pn16q (fp16)
                            nc.tensor.matmul(
                                cp[:, c * 512 : c * 512 + 512],
                                lhsT=onesr16_t[:],
                                rhs=pn16q_t[:, o : o + 512],
                                start=True,
                                stop=False,
                                skip_group_check=True,
                            )
                        xhi_s, xlo_s = x_tiles[s]
                        for half in (xhi_s, xlo_s):
                            for c in range(PH // 512):
                                o = ph * PH + c * 512
                                # cross: K=256 in one DoubleRow fp8 instruction
                                nc.tensor.matmul(
                                    cp[:, c * 512 : c * 512 + 512],
                                    lhsT=half[:, :, qt8 * 128 : qt8 * 128 + 128],
                                    rhs=y8_t[:, :, o : o + 512],
                                    start=False,
                                    stop=(half is xlo_s),
                                    perf_mode=DR,
                                    skip_group_check=True,
                                )
                        # dist straight into macc for s=0, else via a rotating
                        # fp32 tile + exact DVE add (macc must be exact fp32:
                        # a PE f32r accumulation measured 1e-4 rel rounding,
                        # which the variance amplifies 360x -> std absmax 2.7)
                        dst = (
                            macc_t
                            if s == 0
                            else distpool.tile(
                                [128, PH], F32, tag="dist", name=f"d{qt8}_{ph}_{s}"
                            )
                        )
                        nc.scalar.activation(
                            dst[:], cp[:], AF.Sqrt,
                            bias=qncol_t[:, qt8, s : s + 1],
                            scale=1.0,
                        )
                        if s > 0:
                            nc.vector.tensor_add(macc_t[:], macc_t[:], dst[:])
                    # ss = 10*pn + xsum.proto^T (fp16 cross, consistent)
                    ssp = ppC.tile([128, PH], F32, tag="ps", name=f"ss{qt8}_{ph}")
                    for c in range(PH // 512):
                        o = ph * PH + c * 512
                        nc.tensor.matmul(
                            ssp[:, c * 512 : c * 512 + 512],
                            lhsT=onesr16_t[:],
                            rhs=pn10_t[:, o : o + 512],
                            start=True,
                            stop=False,
                            skip_group_check=True,
                        )
                    for et in range(ET):
                        for c in range(PH // 512):
                            o = ph * PH + c * 512
                            nc.tensor.matmul(
                                ssp[:, c * 512 : c * 512 + 512],
                                lhsT=xsum16_t[:, et, qt8 * 128 : qt8 * 128 + 128],
                                rhs=yT16_t[:, et, o : o + 512],
                                start=False,
                                stop=(et == ET - 1),
                                skip_group_check=True,
                            )
                    # omean = macc/S; m2 = omean^2; u = ss - m2*S;
                    # std = Sqrt(u/(S-1) + qnsum/(S-1)) via the ACT bias
                    omean_t = outpool.tile([128, PH], F32, tag="out", name=f"om{qt8}_{ph}")
                    nc.vector.tensor_scalar_mul(omean_t[:], macc_t[:], COMP / S)
                    m2_t = finpool.tile([128, PH], F32, tag="fin", name=f"m2{qt8}_{ph}")
                    nc.vector.tensor_tensor(m2_t[:], omean_t[:], omean_t[:], ALU.mult)
                    u_t = finpool.tile([128, PH], F32, tag="fin", name=f"u{qt8}_{ph}")
                    nc.vector.scalar_tensor_tensor(
                        u_t[:], m2_t[:], -float(S), ssp[:], ALU.mult, ALU.add
                    )
                    ostd_t = outpool.tile([128, PH], F32, tag="out", name=f"os{qt8}_{ph}")
                    nc.scalar.activation(
                        ostd_t[:], u_t[:], AF.Sqrt,
                        bias=qn9_t[:, qt8 : qt8 + 1],
                        scale=1.0 / (S - 1),
                    )
                    nc.sync.dma_start(
                        std_o[qt8 * 128 : qt8 * 128 + 128, ph * PH : ph * PH + PH],
                        ostd_t[:],
                    )
                    nc.sync.dma_start(
                        mean_o[qt8 * 128 : qt8 * 128 + 128, ph * PH : ph * PH + PH],
                        omean_t[:],
                    )


def _prep_inputs(query_features, prototypes, weight_mu, weight_rho, bias_mu, bias_rho, eps_w, eps_b):
    f32, f16 = np.float32, np.float16
    sp_w = np.log1p(np.exp(weight_rho.astype(np.float64))).astype(f32)
    sp_b = np.log1p(np.exp(bias_rho.astype(np.float64))).astype(f32)
    W = (weight_mu[None] + eps_w * sp_w[None]).astype(f32)  # [S,D,D]
    B = (bias_mu[None] + eps_b * sp_b[None]).astype(f32)  # [S,D]
    Wh = W.astype(f16)
    qfh = query_features.astype(f16)  # [Q,D]

    # prototypes quantized once to fp8; everything downstream derives from y8
    y8 = prototypes.astype(f32).astype(NP_F8)  # [P,D]
    y8up = y8.astype(f32)
    pn = (y8up ** 2).sum(-1, dtype=f32)  # [P]
    pn8h = (0.5 * pn).astype(NP_F8)
    pnq = 2.0 * pn8h.astype(f32)
    pn10_16 = (float(S) * pnq).astype(f16)[None, :]  # [1,P]
    b2 = (-2.0 * B).astype(f32)  # [S,D]

    W16 = np.ascontiguousarray(
        Wh.reshape(S, DT, 128, 256).transpose(0, 2, 1, 3).reshape(S, 128, DT * 256)
    )
    b2T = np.ascontiguousarray(
        b2.T.reshape(ET, 128, S).transpose(1, 0, 2).reshape(128, ET * S)
    )
    y8T = np.ascontiguousarray(
        y8.T.reshape(ET, 128, P).transpose(1, 0, 2)
    )  # [128, ET, P] fp8
    yT16 = y8T.astype(f16)  # exact upconvert, same layout
    pn16q = pnq.astype(f16)[None, :]  # [1,P]
    common = {
        "W16": W16,
        "b2T": b2T,
        "y8": y8T,
        "yT16": yT16,
        "pn16q": pn16q,
        "pn10_16": pn10_16,
        "onesr16": np.ones((1, 128), f16),
        "o16c": np.ones((128, 1), f16),
        "eye8": np.eye(128, dtype=NP_F8),
    }
    in_maps = []
    for c in range(NCORES):
        qs = qfh[c * QLOC : (c + 1) * QLOC]  # [QLOC, D]
        qT16 = np.ascontiguousarray(
            qs.T.reshape(DT, 128, QLOC).transpose(1, 0, 2).reshape(128, DT * QLOC)
        )
        in_maps.append({"qT16": qT16, **common})
    return in_maps


def kernel(**inputs):
    global LAST_RESULTS
    n_samples = int(inputs.pop("n_samples", S))
    assert n_samples == S, f"kernel hardcodes S={S}, got {n_samples}"
    np_inputs = {
        k: np.asarray(v, dtype=np.float32)
        for k, v in inputs.items()
    }
    in_maps = _prep_inputs(**np_inputs)

    if "nc" not in _CACHE:
        _CACHE["nc"] = _build_bass()
    nc = _CACHE["nc"]

    trace = bool(int(os.environ.get("KERNEL_TRACE", "0")))
    res = bass_utils.run_bass_kernel_spmd(
        nc, in_maps, core_ids=list(range(NCORES)), trace=trace
    )
    LAST_RESULTS = res
    mean = np.concatenate([r["mean_o"] for r in res.results], axis=0)
    std = np.concatenate([r["std_o"] for r in res.results], axis=0)
    return mean, std
